# revision 1
# baseline (speedup 1.0000x reference)
"""Trainium2 Bass kernel for nn_BinaryDiff: out = x @ base + coeff * (x @ mask).

Fused as a single matmul: out = x @ W where W = base + coeff * mask.

Sharding over 8 NeuronCores: 4 row-groups of x (2048 rows each) x 2
column-groups of W (2048 cols each). Each core computes a [2048, 2048]
tile of the [8192, 4096] output.

Per-core device kernel (two N-half passes to hide the 64 MiB W load):
  - W = base + c*mask built on-chip (one DVE scalar_tensor_tensor per
    k-tile, int32 mask consumed directly), cached in SBUF as bf16.
    Half A (cols 0:N/2) is built up front. Half B's builds are all
    emitted inside PASS A (so read-after-write deps exist before PASS B
    consumes them) with explicit pacing deps onto late-PASS-A output
    copybacks, so its 32 MiB of HBM traffic drains in PASS A's back
    half where bandwidth is free.
  - Each pass, per m-tile: stage x rows fp32, cast to bf16 on ScalarE,
    transpose 128x128 blocks on TensorE (bf16 PSUM, groups of 8 = one
    PSUM bank per merged DVE copyback), then 512-wide bf16 matmuls
    accumulating fp32 in PSUM over K.
  - Output copyback on ScalarE; fp32 out.
"""

import numpy as np

import concourse.bass as bass
import concourse.mybir as mybir
import concourse.tile as tile
from concourse import bacc
from concourse.masks import make_identity

P = 128
FULL_M, FULL_K, FULL_N = 8192, 4096, 4096
ROW_SHARDS, COL_SHARDS = 4, 2
CORE_M = FULL_M // ROW_SHARDS   # 2048
CORE_N = FULL_N // COL_SHARDS   # 2048


def build_kernel(M=CORE_M, K=FULL_K, N=CORE_N, debug=False):
    """Build the per-core Bass program. All cores run the same program (SPMD)."""
    f32 = mybir.dt.float32
    i32 = mybir.dt.int32
    bf16 = mybir.dt.bfloat16

    M_T = M // P            # m-tiles of 128 rows
    K_T = K // P            # k-tiles of 128
    N_MM = 512              # matmul moving free dim (one PSUM bank)
    NH = N // 2             # N half width
    NH_T = NH // N_MM       # 512-subtiles per half
    XH = min(K, 1024)       # x staging chunk width
    XH_T = K // XH

    nc = bacc.Bacc("TRN2", target_bir_lowering=False, debug=debug)

    x_d = nc.dram_tensor("x", [M, K], f32, kind="ExternalInput").ap()
    base_d = nc.dram_tensor("base", [K, N], f32, kind="ExternalInput").ap()
    mask_d = nc.dram_tensor("mask", [K, N], i32, kind="ExternalInput").ap()
    coeff_d = nc.dram_tensor("coeff", [P, 1], f32, kind="ExternalInput").ap()
    out_d = nc.dram_tensor("out", [M, N], f32, kind="ExternalOutput").ap()

    with tile.TileContext(nc) as tc:
        with (
            tc.tile_pool(name="const", bufs=1) as const,
            tc.tile_pool(name="wcache", bufs=1) as wcache,
            tc.tile_pool(name="wstage", bufs=3) as wstage,
            tc.tile_pool(name="xstage", bufs=2) as xstage,
            tc.tile_pool(name="xb", bufs=3) as xbpool,
            tc.tile_pool(name="xt", bufs=3) as xtpool,
            tc.tile_pool(name="ostage", bufs=2) as ostage,
            tc.tile_pool(name="tpsum", bufs=2, space="PSUM") as tpsum,
            tc.tile_pool(name="mpsum", bufs=6, space="PSUM") as mpsum,
        ):
            ident = const.tile([P, P], bf16)
            make_identity(nc, ident[:])
            c128 = const.tile([P, 1], f32)
            nc.sync.dma_start(out=c128[:], in_=coeff_d[:])

            w_a = wcache.tile([P, K_T, NH], bf16, name="w_a")
            w_b = wcache.tile([P, K_T, NH], bf16, name="w_b")

            def build_w_chunk(k, half, anchor=None):
                """Load base/mask k-tile for one N-half and fuse into W.

                `anchor`: optional instruction this chunk's DMAs must wait
                for — used to pace W-half-B's HBM traffic into the back half
                of PASS A so it doesn't crowd out PASS A's own loads."""
                cs = slice(half * NH, (half + 1) * NH)
                dst = w_a if half == 0 else w_b
                bst = wstage.tile([P, NH], f32, name="bst")
                mst = wstage.tile([P, NH], i32, name="mst")
                d1 = nc.sync.dma_start(out=bst[:], in_=base_d[k * P:(k + 1) * P, cs])
                d2 = nc.sync.dma_start(out=mst[:], in_=mask_d[k * P:(k + 1) * P, cs])
                if anchor is not None:
                    tile.add_dep_helper(d1.ins, anchor.ins, reason="pace W-B load")
                    tile.add_dep_helper(d2.ins, anchor.ins, reason="pace W-B load")
                nc.vector.scalar_tensor_tensor(
                    out=dst[:, k, :],
                    in0=mst[:],
                    scalar=c128[:, 0:1],
                    in1=bst[:],
                    op0=mybir.AluOpType.mult,
                    op1=mybir.AluOpType.add,
                )

            TG = min(8, K_T)          # transposes per merged copyback group
                                      # ([P, 8, P] bf16 = exactly one PSUM bank)
            k_groups = [
                list(range(g, min(g + TG, K_T))) for g in range(0, K_T, TG)
            ]

            XSW = min(K, 2 * XH)      # x staging DMA width (bigger DMA rows
                                      # halve the descriptor count; x rows
                                      # are contiguous in DRAM)

            def emit_x_dma(m, reverse=False):
                """Stage x rows fp32 in wide DMAs, cast to bf16 on ScalarE in
                XH-wide slices; return bf16 chunk tiles (transposing bf16
                runs 2x faster on the PE)."""
                rs = slice(m * P, (m + 1) * P)
                chunks = [None] * XH_T
                for hs in range(K // XSW):
                    xst = xstage.tile([P, XSW], f32, name="xst")
                    nc.gpsimd.dma_start(
                        out=xst[:], in_=x_d[rs, hs * XSW:(hs + 1) * XSW]
                    )
                    for sub in range(XSW // XH):
                        h = hs * (XSW // XH) + sub
                        xb = xbpool.tile([P, XH], bf16, name="xb")
                        nc.scalar.copy(
                            out=xb[:], in_=xst[:, sub * XH:(sub + 1) * XH]
                        )
                        chunks[h] = xb
                return chunks

            def emit_t_group(chunks, xt, group):
                """PE-transpose a group of k-tiles, one merged copyback."""
                pst = tpsum.tile([P, TG, P], bf16)
                for j, k in enumerate(group):
                    h, kk = divmod(k, XH // P)
                    nc.tensor.transpose(
                        pst[:, j, :],
                        chunks[h][:, kk * P:(kk + 1) * P],
                        ident[:],
                    )
                g0 = group[0]
                nc.vector.tensor_copy(
                    out=xt[:, g0:g0 + len(group), :], in_=pst[:, :len(group), :]
                )

            def emit_mm_group(xt, w_half, psums, group, k_first, k_last):
                for k in group:
                    for n in range(NH_T):
                        nc.tensor.matmul(
                            psums[n][:],
                            lhsT=xt[:, k, :],
                            rhs=w_half[:, k, n * N_MM:(n + 1) * N_MM],
                            start=(k == k_first),
                            stop=(k == k_last),
                        )

            def emit_out(psums, m, half):
                rs = slice(m * P, (m + 1) * P)
                cps = []
                for n in range(NH_T):
                    ob = ostage.tile([P, N_MM], f32, name="ob")
                    cps.append(nc.scalar.copy(out=ob[:], in_=psums[n][:]))
                    col0 = half * NH + n * N_MM
                    nc.scalar.dma_start(
                        out=out_d[rs, col0:col0 + N_MM], in_=ob[:]
                    )
                return cps

            # ---- W half A up front (k-forward everywhere) ----
            for k in range(K_T):
                build_w_chunk(k, 0)

            # W half B: every chunk is EMITTED inside PASS A (so the
            # STT->matmul read-after-write deps exist), but its DMAs carry an
            # explicit dependency on a late-PASS-A output copyback, pacing
            # the 32 MiB of W-B HBM traffic into PASS A's back half where
            # bandwidth is free. Chunk k's anchor m-tile rises with k so the
            # drain is spread smoothly and finishes just before PASS B
            # consumes it (k-forward).
            wb_m0 = max(0, (M_T * 3) // 8)
            wb_span = max(1, M_T - wb_m0)

            def wb_anchor_m(k):
                return wb_m0 + (k * wb_span) // K_T

            # ---- PASS A: transpose x, matmul vs W-half-A ----
            pending_wb = {m: [] for m in range(M_T)}
            for k in range(K_T):
                pending_wb[wb_anchor_m(k)].append(k)
            for m in range(M_T):
                chunks = emit_x_dma(m)
                xt = xtpool.tile([P, K_T, P], bf16, name="xt")
                for g in k_groups:
                    emit_t_group(chunks, xt, g)
                psums = [
                    mpsum.tile([P, N_MM], f32, name="mmps") for _ in range(NH_T)
                ]
                for g in k_groups:
                    emit_mm_group(xt, w_a, psums, g, 0, K_T - 1)
                cps = emit_out(psums, m, 0)
                for k in pending_wb[m]:
                    build_w_chunk(k, 1, anchor=cps[0])

            # ---- PASS B: re-transpose x, matmul vs W-half-B ----
            for m in range(M_T):
                chunks = emit_x_dma(m)
                xt = xtpool.tile([P, K_T, P], bf16, name="xt")
                for g in k_groups:
                    emit_t_group(chunks, xt, g)
                psums = [
                    mpsum.tile([P, N_MM], f32, name="mmps") for _ in range(NH_T)
                ]
                for g in k_groups:
                    emit_mm_group(xt, w_b, psums, g, 0, K_T - 1)
                emit_out(psums, m, 1)

    nc.compile()
    return nc


_NC_CACHE = {}


def _get_nc():
    if "nc" not in _NC_CACHE:
        _NC_CACHE["nc"] = build_kernel()
    return _NC_CACHE["nc"]


def make_in_maps(x, base, coeff, mask):
    x = np.asarray(x, dtype=np.float32)
    base = np.asarray(base, dtype=np.float32)
    mask = np.asarray(mask, dtype=np.int32)
    coeff = np.asarray(coeff, dtype=np.float32)

    B, L, D_IN = x.shape
    x2 = np.ascontiguousarray(x.reshape(B * L, D_IN))
    c128 = np.full((P, 1), coeff[0], dtype=np.float32)

    in_maps = []
    for i in range(8):
        rg, cg = i // COL_SHARDS, i % COL_SHARDS
        in_maps.append(
            {
                "x": x2[rg * CORE_M:(rg + 1) * CORE_M],
                "base": np.ascontiguousarray(
                    base[:, cg * CORE_N:(cg + 1) * CORE_N]
                ),
                "mask": np.ascontiguousarray(
                    mask[:, cg * CORE_N:(cg + 1) * CORE_N]
                ),
                "coeff": c128,
            }
        )
    return in_maps, (B, L)


def assemble(results, B, L):
    out = np.empty((B * L, FULL_N), dtype=np.float32)
    for i in range(8):
        rg, cg = i // COL_SHARDS, i % COL_SHARDS
        out[rg * CORE_M:(rg + 1) * CORE_M, cg * CORE_N:(cg + 1) * CORE_N] = (
            results[i]["out"]
        )
    return out.reshape(B, L, FULL_N)


def kernel(x, base, coeff, mask):
    from concourse.bass_utils import run_bass_kernel_spmd

    in_maps, (B, L) = make_in_maps(x, base, coeff, mask)
    nc = _get_nc()
    res = run_bass_kernel_spmd(nc, in_maps, list(range(8)))
    return assemble(res.results, B, L)



# revision 2
# speedup vs baseline: 1.3220x; 1.3220x over previous
"""Trainium2 Bass kernel for nn_BinaryDiff: out = x @ base + coeff * (x @ mask).

Fused as a single matmul: out = x @ W where W = base + coeff * mask.

Sharding over 8 NeuronCores: data-parallel over rows — each core computes
1024 rows x 4096 cols of the [8192, 4096] output (x sharded by rows, W
replicated). x is pre-transposed/cast to bf16 on the host as part of the
sharding/layout prep, so the device program is a pure matmul pipeline:

  - x^T shard ([4096, 1024] bf16, 8 MiB) loaded once, resident in SBUF.
  - W streamed in eight 512-col panels: per k-slab, DMA bf16 base + int8
    mask, fuse W = base + c*mask on DVE into a bf16 panel cache
    (double-buffered; panel np+1 builds while panel np's matmuls run).
  - Per panel: 8 PSUM banks (one per 128-row m-tile), 32 k-slabs of
    back-to-back 512-wide bf16 matmuls — PE never transposes, never
    idles, stays HAM-warm.
  - PSUM -> SBUF on ScalarE, fp32 out DMA on the ACT HWDGE ring.
"""

import numpy as np
import ml_dtypes

import concourse.mybir as mybir
import concourse.tile as tile
from concourse import bacc

P = 128
FULL_M, FULL_K, FULL_N = 8192, 4096, 4096
N_CORES = 8
CORE_M = FULL_M // N_CORES      # 1024 rows per core
K_T = FULL_K // P               # 32 k-slabs
M_T = CORE_M // P               # 8 m-tiles
NPAN = 8                        # W panels across N
N_MM = FULL_N // NPAN           # 512 (one PSUM bank)


def build_kernel(debug=False):
    """Build the per-core Bass program. All cores run the same program (SPMD)."""
    f32 = mybir.dt.float32
    i8 = mybir.dt.int8
    bf16 = mybir.dt.bfloat16

    nc = bacc.Bacc("TRN2", target_bir_lowering=False, debug=debug)

    xT_d = nc.dram_tensor("xT", [FULL_K, CORE_M], bf16, kind="ExternalInput").ap()
    # base/mask pretiled on host to [NPAN, K_T, P, N_MM] (flattened rows)
    base_d = nc.dram_tensor(
        "baseT", [NPAN * K_T * P, N_MM], bf16, kind="ExternalInput"
    ).ap()
    mask_d = nc.dram_tensor(
        "maskT", [NPAN * K_T * P, N_MM], i8, kind="ExternalInput"
    ).ap()
    coeff_d = nc.dram_tensor("coeff", [P, 1], f32, kind="ExternalInput").ap()
    out_d = nc.dram_tensor("out", [CORE_M, FULL_N], f32, kind="ExternalOutput").ap()

    with tile.TileContext(nc) as tc:
        with (
            tc.tile_pool(name="const", bufs=1) as const,
            tc.tile_pool(name="xts", bufs=1) as xpool,
            tc.tile_pool(name="wstage", bufs=8) as wstage,
            tc.tile_pool(name="wp", bufs=2) as wpool,
            tc.tile_pool(name="ostage", bufs=4) as ostage,
            tc.tile_pool(name="mpsum", bufs=8, space="PSUM") as mpsum,
        ):
            c128 = const.tile([P, 1], f32)
            nc.sync.dma_start(out=c128[:], in_=coeff_d[:])

            # Resident x^T: [128(k), kt, m] bf16 — 64 KiB/partition.
            xts = xpool.tile([P, K_T, CORE_M], bf16, name="xts")
            for kt in range(K_T):
                nc.gpsimd.dma_start(
                    out=xts[:, kt, :], in_=xT_d[kt * P:(kt + 1) * P, :]
                )

            def build_panel(np_):
                """DMA base/mask k-slabs of one 512-col panel, fuse on DVE."""
                wp = wpool.tile([P, K_T, N_MM], bf16, name="wp")
                for kt in range(K_T):
                    r0 = (np_ * K_T + kt) * P
                    bst = wstage.tile([P, N_MM], bf16, name="bst")
                    mst = wstage.tile([P, N_MM], i8, name="mst")
                    nc.sync.dma_start(out=bst[:], in_=base_d[r0:r0 + P, :])
                    nc.sync.dma_start(out=mst[:], in_=mask_d[r0:r0 + P, :])
                    nc.vector.scalar_tensor_tensor(
                        out=wp[:, kt, :],
                        in0=mst[:],
                        scalar=c128[:, 0:1],
                        in1=bst[:],
                        op0=mybir.AluOpType.mult,
                        op1=mybir.AluOpType.add,
                    )
                return wp

            for np_ in range(NPAN):
                wp = build_panel(np_)
                psums = [
                    mpsum.tile([P, N_MM], f32, name="mmps") for _ in range(M_T)
                ]
                for kt in range(K_T):
                    for m in range(M_T):
                        nc.tensor.matmul(
                            psums[m][:],
                            lhsT=xts[:, kt, m * P:(m + 1) * P],
                            rhs=wp[:, kt, :],
                            start=(kt == 0),
                            stop=(kt == K_T - 1),
                        )
                for m in range(M_T):
                    ob = ostage.tile([P, N_MM], f32, name="ob")
                    nc.scalar.copy(out=ob[:], in_=psums[m][:])
                    nc.scalar.dma_start(
                        out=out_d[
                            m * P:(m + 1) * P, np_ * N_MM:(np_ + 1) * N_MM
                        ],
                        in_=ob[:],
                    )

    nc.compile()
    return nc


_NC_CACHE = {}


def _get_nc():
    if "nc" not in _NC_CACHE:
        _NC_CACHE["nc"] = build_kernel()
    return _NC_CACHE["nc"]


def make_in_maps(x, base, coeff, mask):
    bf16 = ml_dtypes.bfloat16
    x2 = np.asarray(x, np.float32).reshape(FULL_M, FULL_K)

    # W inputs pretiled to [NPAN, K_T, P, N_MM] so every k-slab DMA is a
    # single contiguous block. Shared by all 8 cores (W is replicated).
    baseT = np.ascontiguousarray(
        np.asarray(base, np.float32).astype(bf16)
        .reshape(K_T, P, NPAN, N_MM).transpose(2, 0, 1, 3)
    ).reshape(NPAN * K_T * P, N_MM)
    maskT = np.ascontiguousarray(
        np.asarray(mask).astype(np.int8)
        .reshape(K_T, P, NPAN, N_MM).transpose(2, 0, 1, 3)
    ).reshape(NPAN * K_T * P, N_MM)
    c128 = np.full((P, 1), np.asarray(coeff, np.float32)[0], np.float32)

    in_maps = []
    for i in range(N_CORES):
        xc = x2[i * CORE_M:(i + 1) * CORE_M, :].astype(bf16)
        in_maps.append(
            {
                "xT": np.ascontiguousarray(xc.T),
                "baseT": baseT,
                "maskT": maskT,
                "coeff": c128,
            }
        )
    return in_maps, x.shape[:2]


def assemble(results, B, L):
    out = np.concatenate([results[i]["out"] for i in range(N_CORES)], axis=0)
    return out.reshape(B, L, FULL_N)


def kernel(x, base, coeff, mask):
    from concourse.bass_utils import run_bass_kernel_spmd

    in_maps, (B, L) = make_in_maps(x, base, coeff, mask)
    nc = _get_nc()
    res = run_bass_kernel_spmd(nc, in_maps, list(range(8)))
    return assemble(res.results, B, L)


# revision 3
# speedup vs baseline: 1.3268x; 1.0036x over previous
"""Trainium2 Bass kernel for nn_BinaryDiff: out = x @ base + coeff * (x @ mask).

Fused as a single matmul: out = x @ W where W = base + coeff * mask.

Sharding over 8 NeuronCores: data-parallel over rows — each core computes
1024 rows x 4096 cols of the [8192, 4096] output (x sharded by rows, W
replicated). x is pre-transposed/cast to bf16 on the host as part of the
sharding/layout prep, so the device program is a pure matmul pipeline:

  - x^T shard ([4096, 1024] bf16, 8 MiB) loaded once, resident in SBUF
    (SWDGE ring).
  - W streamed in eight 512-col panels, p-major host layout so DMA lines
    are 8 KiB (base bf16) / 4 KiB (mask int8) per partition; base chunks
    on the SP HWDGE ring, mask chunks on the ACT HWDGE ring (parallel
    rings so W prefetch runs a full panel ahead). Fused W = base + c*mask
    on DVE into a double-buffered bf16 panel cache.
  - Per panel: 8 PSUM banks (one per 128-row m-tile), 32 k-slabs of
    back-to-back 512-wide bf16 matmuls — PE never transposes, never
    idles, stays HAM-warm.
  - PSUM drained by alternating ScalarE/VectorE copies (halves the
    bank-handoff serialization at panel boundaries); fp32 out DMA on the
    SWDGE ring. Panel np+2's W build is emitted before panel np's drain
    so the DVE FIFO never delays W prefetch.
"""

import numpy as np
import ml_dtypes

import concourse.mybir as mybir
import concourse.tile as tile
from concourse import bacc

P = 128
FULL_M, FULL_K, FULL_N = 8192, 4096, 4096
N_CORES = 8
CORE_M = FULL_M // N_CORES      # 1024 rows per core
K_T = FULL_K // P               # 32 k-slabs
M_T = CORE_M // P               # 8 m-tiles
NPAN = 8                        # W panels across N
N_MM = FULL_N // NPAN           # 512 (one PSUM bank)
CH = 8                          # k-slabs per W DMA chunk
N_CH = K_T // CH                # 4 chunks per panel


def build_kernel(debug=False):
    """Build the per-core Bass program. All cores run the same program (SPMD)."""
    f32 = mybir.dt.float32
    i8 = mybir.dt.int8
    bf16 = mybir.dt.bfloat16

    nc = bacc.Bacc("TRN2", target_bir_lowering=False, debug=debug)

    xT_d = nc.dram_tensor("xT", [FULL_K, CORE_M], bf16, kind="ExternalInput").ap()
    # base/mask pretiled on host to [NPAN, P, K_T, N_MM] (p-major panels,
    # flattened to 2D) so per-partition DMA lines are contiguous.
    base_d = nc.dram_tensor(
        "baseT", [NPAN * P, K_T * N_MM], bf16, kind="ExternalInput"
    ).ap()
    mask_d = nc.dram_tensor(
        "maskT", [NPAN * P, K_T * N_MM], i8, kind="ExternalInput"
    ).ap()
    coeff_d = nc.dram_tensor("coeff", [P, 1], f32, kind="ExternalInput").ap()
    out_d = nc.dram_tensor("out", [CORE_M, FULL_N], f32, kind="ExternalOutput").ap()

    with tile.TileContext(nc) as tc:
        with (
            tc.tile_pool(name="const", bufs=1) as const,
            tc.tile_pool(name="xts", bufs=1) as xpool,
            tc.tile_pool(name="bstage", bufs=3) as bstage,
            tc.tile_pool(name="mstage", bufs=3) as mstage,
            tc.tile_pool(name="wp", bufs=2) as wpool,
            tc.tile_pool(name="ostage", bufs=8) as ostage,
            tc.tile_pool(name="mpsum", bufs=8, space="PSUM") as mpsum,
        ):
            c128 = const.tile([P, 1], f32)
            nc.sync.dma_start(out=c128[:], in_=coeff_d[:])

            # Resident x^T: [128(k), kt, m] bf16 — 64 KiB/partition.
            xts = xpool.tile([P, K_T, CORE_M], bf16, name="xts")
            for kt in range(K_T):
                nc.gpsimd.dma_start(
                    out=xts[:, kt, :], in_=xT_d[kt * P:(kt + 1) * P, :]
                )

            def build_panel(np_):
                """DMA base/mask chunks of one 512-col panel, fuse on DVE."""
                wp = wpool.tile([P, K_T, N_MM], bf16, name="wp")
                rs = slice(np_ * P, (np_ + 1) * P)
                for c in range(N_CH):
                    cs = slice(c * CH * N_MM, (c + 1) * CH * N_MM)
                    bst = bstage.tile([P, CH, N_MM], bf16, name="bst")
                    mst = mstage.tile([P, CH, N_MM], i8, name="mst")
                    nc.sync.dma_start(out=bst[:], in_=base_d[rs, cs])
                    nc.scalar.dma_start(out=mst[:], in_=mask_d[rs, cs])
                    for j in range(CH):
                        nc.vector.scalar_tensor_tensor(
                            out=wp[:, c * CH + j, :],
                            in0=mst[:, j, :],
                            scalar=c128[:, 0:1],
                            in1=bst[:, j, :],
                            op0=mybir.AluOpType.mult,
                            op1=mybir.AluOpType.add,
                        )
                return wp

            wps = {0: build_panel(0), 1: build_panel(1)}
            for np_ in range(NPAN):
                wp = wps.pop(np_)
                psums = [
                    mpsum.tile([P, N_MM], f32, name="mmps") for _ in range(M_T)
                ]
                for kt in range(K_T):
                    for m in range(M_T):
                        nc.tensor.matmul(
                            psums[m][:],
                            lhsT=xts[:, kt, m * P:(m + 1) * P],
                            rhs=wp[:, kt, :],
                            start=(kt == 0),
                            stop=(kt == K_T - 1),
                        )
                if np_ + 2 < NPAN:
                    wps[np_ + 2] = build_panel(np_ + 2)
                for m in range(M_T):
                    ob = ostage.tile([P, N_MM], f32, name="ob")
                    if m % 2 == 0:
                        nc.scalar.copy(out=ob[:], in_=psums[m][:])
                    else:
                        nc.vector.tensor_copy(out=ob[:], in_=psums[m][:])
                    nc.gpsimd.dma_start(
                        out=out_d[
                            m * P:(m + 1) * P, np_ * N_MM:(np_ + 1) * N_MM
                        ],
                        in_=ob[:],
                    )

    nc.compile()
    return nc


_NC_CACHE = {}


def _get_nc():
    if "nc" not in _NC_CACHE:
        _NC_CACHE["nc"] = build_kernel()
    return _NC_CACHE["nc"]


def make_in_maps(x, base, coeff, mask):
    bf16 = ml_dtypes.bfloat16
    x2 = np.asarray(x, np.float32).reshape(FULL_M, FULL_K)

    # W inputs pretiled to [NPAN, P, K_T, N_MM] (p-major panels) so each
    # partition's panel data is one contiguous DMA line. Shared by all 8
    # cores (W is replicated).
    baseT = np.ascontiguousarray(
        np.asarray(base, np.float32).astype(bf16)
        .reshape(K_T, P, NPAN, N_MM).transpose(2, 1, 0, 3)
    ).reshape(NPAN * P, K_T * N_MM)
    maskT = np.ascontiguousarray(
        np.asarray(mask).astype(np.int8)
        .reshape(K_T, P, NPAN, N_MM).transpose(2, 1, 0, 3)
    ).reshape(NPAN * P, K_T * N_MM)
    c128 = np.full((P, 1), np.asarray(coeff, np.float32)[0], np.float32)

    in_maps = []
    for i in range(N_CORES):
        xc = x2[i * CORE_M:(i + 1) * CORE_M, :].astype(bf16)
        in_maps.append(
            {
                "xT": np.ascontiguousarray(xc.T),
                "baseT": baseT,
                "maskT": maskT,
                "coeff": c128,
            }
        )
    return in_maps, x.shape[:2]


def assemble(results, B, L):
    out = np.concatenate([results[i]["out"] for i in range(N_CORES)], axis=0)
    return out.reshape(B, L, FULL_N)


def kernel(x, base, coeff, mask):
    from concourse.bass_utils import run_bass_kernel_spmd

    in_maps, (B, L) = make_in_maps(x, base, coeff, mask)
    nc = _get_nc()
    res = run_bass_kernel_spmd(nc, in_maps, list(range(8)))
    return assemble(res.results, B, L)


# revision 5
# speedup vs baseline: 1.3698x; 1.0324x over previous
"""Trainium2 Bass kernel for nn_BinaryDiff: out = x @ base + coeff * (x @ mask).

Fused as a single matmul: out = x @ W where W = base + coeff * mask.

Sharding over 8 NeuronCores: data-parallel over rows — each core computes
1024 rows x 4096 cols of the [8192, 4096] output (x sharded by rows, W
replicated). x is pre-transposed/cast to bf16 on the host as part of the
sharding/layout prep, so the device program is a pure matmul pipeline:

  - x^T shard ([4096, 1024] bf16, 8 MiB) loaded once, resident in SBUF
    (SWDGE ring).
  - W streamed in eight 512-col panels, p-major host layout so DMA lines
    are 8 KiB (base bf16) / 4 KiB (mask int8) per partition; base chunks
    on the SP HWDGE ring, mask chunks on the ACT HWDGE ring (parallel
    rings so W prefetch runs a full panel ahead). Fused W = base + c*mask
    on DVE into a double-buffered bf16 panel cache.
  - A short burst of warm-up matmuls on a memset tile runs while the
    first W chunks land, so the PE HAM clock-gate is already at 8/8 when
    real work starts (otherwise the whole startup runs at 1.2 GHz and
    oscillates).
  - Per panel: two waves of 4 m-tiles, rotating through the 8 PSUM banks
    — a wave's drains overlap the next wave's matmuls, so there is no
    bank-handoff stall at panel boundaries and the final drain tail is
    half as deep.
  - PSUM drained by alternating ScalarE/VectorE copies; fp32 out DMAs
    alternate between the SWDGE and SP rings.
"""

import numpy as np
import ml_dtypes

import concourse.mybir as mybir
import concourse.tile as tile
from concourse import bacc

P = 128
FULL_M, FULL_K, FULL_N = 8192, 4096, 4096
N_CORES = 8
CORE_M = FULL_M // N_CORES      # 1024 rows per core
K_T = FULL_K // P               # 32 k-slabs
M_T = CORE_M // P               # 8 m-tiles
NPAN = 8                        # W panels across N
N_MM = FULL_N // NPAN           # 512 (one PSUM bank)
CH = 8                          # k-slabs per W DMA chunk
N_CH = K_T // CH                # 4 chunks per panel
WAVE = 4                        # m-tiles per PSUM wave
N_WARM = 24                     # HAM warm-up matmuls


def build_kernel(debug=False):
    """Build the per-core Bass program. All cores run the same program (SPMD)."""
    f32 = mybir.dt.float32
    i8 = mybir.dt.int8
    bf16 = mybir.dt.bfloat16

    nc = bacc.Bacc("TRN2", target_bir_lowering=False, debug=debug)

    xT_d = nc.dram_tensor("xT", [FULL_K, CORE_M], bf16, kind="ExternalInput").ap()
    # base/mask pretiled on host to [NPAN, P, K_T, N_MM] (p-major panels,
    # flattened to 2D) so per-partition DMA lines are contiguous.
    base_d = nc.dram_tensor(
        "baseT", [NPAN * P, K_T * N_MM], bf16, kind="ExternalInput"
    ).ap()
    mask_d = nc.dram_tensor(
        "maskT", [NPAN * P, K_T * N_MM], i8, kind="ExternalInput"
    ).ap()
    coeff_d = nc.dram_tensor("coeff", [P, 1], f32, kind="ExternalInput").ap()
    out_d = nc.dram_tensor("out", [CORE_M, FULL_N], f32, kind="ExternalOutput").ap()

    with tile.TileContext(nc) as tc:
        with (
            tc.tile_pool(name="const", bufs=1) as const,
            tc.tile_pool(name="xts", bufs=1) as xpool,
            tc.tile_pool(name="bstage", bufs=3) as bstage,
            tc.tile_pool(name="mstage", bufs=3) as mstage,
            tc.tile_pool(name="wp", bufs=2) as wpool,
            tc.tile_pool(name="ostage", bufs=8) as ostage,
            tc.tile_pool(name="mpsum", bufs=8, space="PSUM") as mpsum,
        ):
            # HAM warm-up: PE runs junk matmuls at cold clock while the
            # first real operands stream in; by the time they land the
            # clock gate is open.
            wu = const.tile([P, 5 * P], bf16)
            nc.vector.memset(wu[:], 0.0)
            warm_ps = mpsum.tile([P, N_MM], f32, name="mmps")
            for _ in range(N_WARM):
                nc.tensor.matmul(
                    warm_ps[:], lhsT=wu[:, :P], rhs=wu[:, P:], start=True,
                    stop=True,
                )

            c128 = const.tile([P, 1], f32)
            nc.scalar.dma_start(out=c128[:], in_=coeff_d[:])

            # Resident x^T: [128(k), kt, m] bf16 — 64 KiB/partition.
            xts = xpool.tile([P, K_T, CORE_M], bf16, name="xts")
            for kt in range(K_T):
                nc.gpsimd.dma_start(
                    out=xts[:, kt, :], in_=xT_d[kt * P:(kt + 1) * P, :]
                )

            def build_panel(np_):
                """DMA base/mask chunks of one 512-col panel, fuse on DVE."""
                wp = wpool.tile([P, K_T, N_MM], bf16, name="wp")
                rs = slice(np_ * P, (np_ + 1) * P)
                for c in range(N_CH):
                    cs = slice(c * CH * N_MM, (c + 1) * CH * N_MM)
                    bst = bstage.tile([P, CH, N_MM], bf16, name="bst")
                    mst = mstage.tile([P, CH, N_MM], i8, name="mst")
                    nc.sync.dma_start(out=bst[:], in_=base_d[rs, cs])
                    nc.scalar.dma_start(out=mst[:], in_=mask_d[rs, cs])
                    for j in range(CH):
                        nc.vector.scalar_tensor_tensor(
                            out=wp[:, c * CH + j, :],
                            in0=mst[:, j, :],
                            scalar=c128[:, 0:1],
                            in1=bst[:, j, :],
                            op0=mybir.AluOpType.mult,
                            op1=mybir.AluOpType.add,
                        )
                return wp

            wps = {0: build_panel(0), 1: build_panel(1)}
            for np_ in range(NPAN):
                wp = wps.pop(np_)
                for w0 in range(0, M_T, WAVE):
                    psums = {
                        m: mpsum.tile([P, N_MM], f32, name="mmps")
                        for m in range(w0, w0 + WAVE)
                    }
                    for kt in range(K_T):
                        for m in range(w0, w0 + WAVE):
                            nc.tensor.matmul(
                                psums[m][:],
                                lhsT=xts[:, kt, m * P:(m + 1) * P],
                                rhs=wp[:, kt, :],
                                start=(kt == 0),
                                stop=(kt == K_T - 1),
                            )
                    for i, m in enumerate(sorted(psums)):
                        ob = ostage.tile([P, N_MM], f32, name="ob")
                        if i % 2 == 0:
                            nc.scalar.copy(out=ob[:], in_=psums[m][:])
                        else:
                            nc.vector.tensor_copy(out=ob[:], in_=psums[m][:])
                        dma_eng = nc.gpsimd if i % 2 == 0 else nc.sync
                        dma_eng.dma_start(
                            out=out_d[
                                m * P:(m + 1) * P, np_ * N_MM:(np_ + 1) * N_MM
                            ],
                            in_=ob[:],
                        )
                if np_ + 2 < NPAN:
                    wps[np_ + 2] = build_panel(np_ + 2)

    nc.compile()
    return nc


_NC_CACHE = {}


def _get_nc():
    if "nc" not in _NC_CACHE:
        _NC_CACHE["nc"] = build_kernel()
    return _NC_CACHE["nc"]


def make_in_maps(x, base, coeff, mask):
    bf16 = ml_dtypes.bfloat16
    x2 = np.asarray(x, np.float32).reshape(FULL_M, FULL_K)

    # W inputs pretiled to [NPAN, P, K_T, N_MM] (p-major panels) so each
    # partition's panel data is one contiguous DMA line. Shared by all 8
    # cores (W is replicated).
    baseT = np.ascontiguousarray(
        np.asarray(base, np.float32).astype(bf16)
        .reshape(K_T, P, NPAN, N_MM).transpose(2, 1, 0, 3)
    ).reshape(NPAN * P, K_T * N_MM)
    maskT = np.ascontiguousarray(
        np.asarray(mask).astype(np.int8)
        .reshape(K_T, P, NPAN, N_MM).transpose(2, 1, 0, 3)
    ).reshape(NPAN * P, K_T * N_MM)
    c128 = np.full((P, 1), np.asarray(coeff, np.float32)[0], np.float32)

    in_maps = []
    for i in range(N_CORES):
        xc = x2[i * CORE_M:(i + 1) * CORE_M, :].astype(bf16)
        in_maps.append(
            {
                "xT": np.ascontiguousarray(xc.T),
                "baseT": baseT,
                "maskT": maskT,
                "coeff": c128,
            }
        )
    return in_maps, x.shape[:2]


def assemble(results, B, L):
    out = np.concatenate([results[i]["out"] for i in range(N_CORES)], axis=0)
    return out.reshape(B, L, FULL_N)


def kernel(x, base, coeff, mask):
    from concourse.bass_utils import run_bass_kernel_spmd

    in_maps, (B, L) = make_in_maps(x, base, coeff, mask)
    nc = _get_nc()
    res = run_bass_kernel_spmd(nc, in_maps, list(range(8)))
    return assemble(res.results, B, L)


# revision 7
# speedup vs baseline: 1.4145x; 1.0327x over previous
"""Trainium2 Bass kernel for nn_BinaryDiff: out = x @ base + coeff * (x @ mask).

Fused as a single matmul: out = x @ W where W = base + coeff * mask.

Sharding over 8 NeuronCores: data-parallel over rows — each core computes
1024 rows x 4096 cols of the [8192, 4096] output (x sharded by rows, W
replicated). x is pre-transposed/cast to bf16 on the host as part of the
sharding/layout prep, so the device program is a pure matmul pipeline:

  - x^T shard ([4096, 1024] bf16, 8 MiB) loaded once, resident in SBUF
    (SWDGE ring).
  - W streamed in eight 512-col panels, p-major host layout so DMA lines
    are 8 KiB (base bf16) / 4 KiB (mask int8) per partition; base chunks
    on the SP HWDGE ring, mask chunks on the ACT HWDGE ring (parallel
    rings so W prefetch runs a full panel ahead). Fused W = base + c*mask
    on DVE into a double-buffered bf16 panel cache.
  - A short burst of warm-up matmuls on a memset tile runs while the
    first W chunks land, so the PE HAM clock-gate is already at 8/8 when
    real work starts (otherwise the whole startup runs at 1.2 GHz and
    oscillates).
  - Per panel: two waves of 4 m-tiles, rotating through the 8 PSUM banks
    — a wave's drains overlap the next wave's matmuls, so there is no
    bank-handoff stall at panel boundaries and the final drain tail is
    half as deep.
  - PSUM drained by alternating ScalarE/VectorE copies; fp32 out DMAs
    alternate between the SWDGE and SP rings.
"""

import numpy as np
import ml_dtypes

import concourse.mybir as mybir
import concourse.tile as tile
from concourse import bacc

P = 128
FULL_M, FULL_K, FULL_N = 8192, 4096, 4096
N_CORES = 8
CORE_M = FULL_M // N_CORES      # 1024 rows per core
K_T = FULL_K // P               # 32 k-slabs
M_T = CORE_M // P               # 8 m-tiles
NPAN = 8                        # W panels across N
N_MM = FULL_N // NPAN           # 512 (one PSUM bank)
CH = 8                          # k-slabs per W DMA chunk
N_CH = K_T // CH                # 4 chunks per panel
WAVE = 4                        # m-tiles per PSUM wave (panels 1+)
N_WARM = 28                     # HAM warm-up matmuls


def build_kernel(debug=False):
    """Build the per-core Bass program. All cores run the same program (SPMD)."""
    f32 = mybir.dt.float32
    i8 = mybir.dt.int8
    bf16 = mybir.dt.bfloat16

    nc = bacc.Bacc("TRN2", target_bir_lowering=False, debug=debug)

    xT_d = nc.dram_tensor("xT", [FULL_K, CORE_M], bf16, kind="ExternalInput").ap()
    # base/mask pretiled on host to [NPAN, P, K_T, N_MM] (p-major panels,
    # flattened to 2D) so per-partition DMA lines are contiguous.
    base_d = nc.dram_tensor(
        "baseT", [NPAN * P, K_T * N_MM], bf16, kind="ExternalInput"
    ).ap()
    mask_d = nc.dram_tensor(
        "maskT", [NPAN * P, K_T * N_MM], i8, kind="ExternalInput"
    ).ap()
    coeff_d = nc.dram_tensor("coeff", [P, 1], f32, kind="ExternalInput").ap()
    out_d = nc.dram_tensor("out", [CORE_M, FULL_N], f32, kind="ExternalOutput").ap()

    with tile.TileContext(nc) as tc:
        with (
            tc.tile_pool(name="const", bufs=1) as const,
            tc.tile_pool(name="xts", bufs=1) as xpool,
            tc.tile_pool(name="bstage", bufs=3) as bstage,
            tc.tile_pool(name="mstage", bufs=3) as mstage,
            tc.tile_pool(name="wp", bufs=2) as wpool,
            tc.tile_pool(name="ostage", bufs=8) as ostage,
            tc.tile_pool(name="mpsum", bufs=8, space="PSUM") as mpsum,
        ):
            # HAM warm-up: PE runs junk matmuls at cold clock while the
            # first real operands stream in; by the time they land the
            # clock gate is open.
            wu = const.tile([P, 5 * P], bf16)
            nc.vector.memset(wu[:], 0.0)
            warm_ps = mpsum.tile([P, N_MM], f32, name="mmps")
            for _ in range(N_WARM):
                nc.tensor.matmul(
                    warm_ps[:], lhsT=wu[:, :P], rhs=wu[:, P:], start=True,
                    stop=True,
                )

            c128 = const.tile([P, 1], f32)
            nc.scalar.dma_start(out=c128[:], in_=coeff_d[:])

            first_chunk = [None]

            def build_panel(np_):
                """DMA base/mask chunks of one 512-col panel, fuse on DVE."""
                wp = wpool.tile([P, K_T, N_MM], bf16, name="wp")
                rs = slice(np_ * P, (np_ + 1) * P)
                for c in range(N_CH):
                    cs = slice(c * CH * N_MM, (c + 1) * CH * N_MM)
                    bst = bstage.tile([P, CH, N_MM], bf16, name="bst")
                    mst = mstage.tile([P, CH, N_MM], i8, name="mst")
                    db = nc.sync.dma_start(out=bst[:], in_=base_d[rs, cs])
                    nc.scalar.dma_start(out=mst[:], in_=mask_d[rs, cs])
                    if first_chunk[0] is None:
                        first_chunk[0] = db
                    for j in range(CH):
                        nc.vector.scalar_tensor_tensor(
                            out=wp[:, c * CH + j, :],
                            in0=mst[:, j, :],
                            scalar=c128[:, 0:1],
                            in1=bst[:, j, :],
                            op0=mybir.AluOpType.mult,
                            op1=mybir.AluOpType.add,
                        )
                return wp

            # Resident x^T: [128(k), kt, m] bf16 — 64 KiB/partition. The
            # first few slabs race the first W chunk; the rest are paced
            # behind it so the panel-0 critical path isn't starved of HBM
            # bandwidth at startup.
            xts = xpool.tile([P, K_T, CORE_M], bf16, name="xts")
            for kt in range(4):
                nc.gpsimd.dma_start(
                    out=xts[:, kt, :], in_=xT_d[kt * P:(kt + 1) * P, :]
                )
            wps = {0: build_panel(0)}
            for kt in range(4, K_T):
                dx = nc.gpsimd.dma_start(
                    out=xts[:, kt, :], in_=xT_d[kt * P:(kt + 1) * P, :]
                )
                tile.add_dep_helper(
                    dx.ins, first_chunk[0].ins, reason="pace x behind W chunk0"
                )
            wps[1] = build_panel(1)

            for np_ in range(NPAN):
                wp = wps.pop(np_)
                # Panel 0 runs as one 8-bank wave: its matmuls are paced by
                # x/W arrival (1.73 us per k-slab consumption keeps PE fed);
                # later panels use 4-tile waves so drains overlap compute
                # with no bank-handoff stall.
                wave = M_T if np_ == 0 else WAVE
                for w0 in range(0, M_T, wave):
                    psums = {
                        m: mpsum.tile([P, N_MM], f32, name="mmps")
                        for m in range(w0, w0 + wave)
                    }
                    for kt in range(K_T):
                        for m in range(w0, w0 + wave):
                            nc.tensor.matmul(
                                psums[m][:],
                                lhsT=xts[:, kt, m * P:(m + 1) * P],
                                rhs=wp[:, kt, :],
                                start=(kt == 0),
                                stop=(kt == K_T - 1),
                            )
                    for i, m in enumerate(sorted(psums)):
                        ob = ostage.tile([P, N_MM], f32, name="ob")
                        if i % 2 == 0:
                            nc.scalar.copy(out=ob[:], in_=psums[m][:])
                        else:
                            nc.vector.tensor_copy(out=ob[:], in_=psums[m][:])
                        dma_eng = nc.sync if i % 2 == 0 else nc.scalar
                        dma_eng.dma_start(
                            out=out_d[
                                m * P:(m + 1) * P, np_ * N_MM:(np_ + 1) * N_MM
                            ],
                            in_=ob[:],
                        )
                if np_ + 2 < NPAN:
                    wps[np_ + 2] = build_panel(np_ + 2)

    nc.compile()
    return nc


_NC_CACHE = {}


def _get_nc():
    if "nc" not in _NC_CACHE:
        _NC_CACHE["nc"] = build_kernel()
    return _NC_CACHE["nc"]


def make_in_maps(x, base, coeff, mask):
    bf16 = ml_dtypes.bfloat16
    x2 = np.asarray(x, np.float32).reshape(FULL_M, FULL_K)

    # W inputs pretiled to [NPAN, P, K_T, N_MM] (p-major panels) so each
    # partition's panel data is one contiguous DMA line. Shared by all 8
    # cores (W is replicated).
    baseT = np.ascontiguousarray(
        np.asarray(base, np.float32).astype(bf16)
        .reshape(K_T, P, NPAN, N_MM).transpose(2, 1, 0, 3)
    ).reshape(NPAN * P, K_T * N_MM)
    maskT = np.ascontiguousarray(
        np.asarray(mask).astype(np.int8)
        .reshape(K_T, P, NPAN, N_MM).transpose(2, 1, 0, 3)
    ).reshape(NPAN * P, K_T * N_MM)
    c128 = np.full((P, 1), np.asarray(coeff, np.float32)[0], np.float32)

    in_maps = []
    for i in range(N_CORES):
        xc = x2[i * CORE_M:(i + 1) * CORE_M, :].astype(bf16)
        in_maps.append(
            {
                "xT": np.ascontiguousarray(xc.T),
                "baseT": baseT,
                "maskT": maskT,
                "coeff": c128,
            }
        )
    return in_maps, x.shape[:2]


def assemble(results, B, L):
    out = np.concatenate([results[i]["out"] for i in range(N_CORES)], axis=0)
    return out.reshape(B, L, FULL_N)


def kernel(x, base, coeff, mask):
    from concourse.bass_utils import run_bass_kernel_spmd

    in_maps, (B, L) = make_in_maps(x, base, coeff, mask)
    nc = _get_nc()
    res = run_bass_kernel_spmd(nc, in_maps, list(range(8)))
    return assemble(res.results, B, L)


# revision 8
# speedup vs baseline: 1.4372x; 1.0161x over previous
"""Trainium2 Bass kernel for nn_BinaryDiff: out = x @ base + coeff * (x @ mask).

Fused as a single matmul: out = x @ W where W = base + coeff * mask.

Sharding over 8 NeuronCores: data-parallel over rows — each core computes
1024 rows x 4096 cols of the [8192, 4096] output (x sharded by rows, W
replicated). x is pre-transposed/cast to bf16 on the host as part of the
sharding/layout prep, so the device program is a pure matmul pipeline:

  - x^T shard ([4096, 1024] bf16, 8 MiB) loaded once, resident in SBUF.
    Slabs alternate between the SWDGE ring and the ACT HWDGE ring
    (interleaved with panel-0 mask chunks in consumption order) — the
    SDMA engines round-robin across queues, so one queue alone gets only
    ~1/3 of HBM bandwidth and panel 0 would starve.
  - W streamed in eight 512-col panels, p-major host layout so DMA lines
    are contiguous per partition; base chunks on the SP HWDGE ring, mask
    chunks on the ACT HWDGE ring. Panel 0 uses 2-slab chunks (fast first
    arrival), later panels 8-slab chunks. Fused W = base + c*mask on DVE
    into a double-buffered bf16 panel cache, prefetched one panel ahead.
  - A short burst of warm-up matmuls on a memset tile runs while the
    first chunks land, so the PE HAM clock-gate is already at 8/8 when
    real work starts.
  - Panel 0 runs as one 8-bank wave (matmul consumption 1.73us/k-slab
    matches DMA delivery); later panels run two 4-bank waves rotating
    through the 8 PSUM banks so drains overlap the next wave's matmuls
    with no bank-handoff stall. The last panel drains in 4/2/2 waves to
    shorten the end-of-kernel DMA tail.
  - PSUM drained by alternating ScalarE/VectorE copies; fp32 out DMAs
    alternate between the two HWDGE rings.
"""

import numpy as np
import ml_dtypes

import concourse.mybir as mybir
import concourse.tile as tile
from concourse import bacc

P = 128
FULL_M, FULL_K, FULL_N = 8192, 4096, 4096
N_CORES = 8
CORE_M = FULL_M // N_CORES      # 1024 rows per core
K_T = FULL_K // P               # 32 k-slabs
M_T = CORE_M // P               # 8 m-tiles
NPAN = 8                        # W panels across N
N_MM = FULL_N // NPAN           # 512 (one PSUM bank)
N_WARM = 14                     # HAM warm-up matmuls


def build_kernel(debug=False):
    """Build the per-core Bass program. All cores run the same program (SPMD)."""
    f32 = mybir.dt.float32
    i8 = mybir.dt.int8
    bf16 = mybir.dt.bfloat16

    nc = bacc.Bacc("TRN2", target_bir_lowering=False, debug=debug)

    xT_d = nc.dram_tensor("xT", [FULL_K, CORE_M], bf16, kind="ExternalInput").ap()
    # base/mask pretiled on host to [NPAN, P, K_T, N_MM] (p-major panels,
    # flattened to 2D) so per-partition DMA lines are contiguous.
    base_d = nc.dram_tensor(
        "baseT", [NPAN * P, K_T * N_MM], bf16, kind="ExternalInput"
    ).ap()
    mask_d = nc.dram_tensor(
        "maskT", [NPAN * P, K_T * N_MM], i8, kind="ExternalInput"
    ).ap()
    coeff_d = nc.dram_tensor("coeff", [P, 1], f32, kind="ExternalInput").ap()
    out_d = nc.dram_tensor("out", [CORE_M, FULL_N], f32, kind="ExternalOutput").ap()

    with tile.TileContext(nc) as tc:
        with (
            tc.tile_pool(name="const", bufs=1) as const,
            tc.tile_pool(name="xts", bufs=1) as xpool,
            tc.tile_pool(name="bstage", bufs=3) as bstage,
            tc.tile_pool(name="mstage", bufs=3) as mstage,
            tc.tile_pool(name="wp", bufs=2) as wpool,
            tc.tile_pool(name="ostage", bufs=8) as ostage,
            tc.tile_pool(name="mpsum", bufs=8, space="PSUM") as mpsum,
        ):
            # HAM warm-up: PE runs junk matmuls at cold clock while the
            # first real operands stream in; by the time they land the
            # clock gate is open.
            wu = const.tile([P, 5 * P], bf16)
            nc.vector.memset(wu[:], 0.0)
            warm_ps = mpsum.tile([P, N_MM], f32, name="mmps")
            for _ in range(N_WARM):
                nc.tensor.matmul(
                    warm_ps[:], lhsT=wu[:, :P], rhs=wu[:, P:], start=True,
                    stop=True,
                )

            c128 = const.tile([P, 1], f32)
            nc.scalar.dma_start(out=c128[:], in_=coeff_d[:])

            xts = xpool.tile([P, K_T, CORE_M], bf16, name="xts")

            def x_slab(kt, eng, anchor=None):
                dx = eng.dma_start(
                    out=xts[:, kt, :], in_=xT_d[kt * P:(kt + 1) * P, :]
                )
                if anchor is not None:
                    tile.add_dep_helper(
                        dx.ins, anchor.ins, reason="pace x behind W chunk0"
                    )
                return dx

            def build_chunk(wp, np_, kt0, ch, stage_tag):
                """DMA one base/mask chunk [kt0, kt0+ch) of panel np_, fuse."""
                rs = slice(np_ * P, (np_ + 1) * P)
                cs = slice(kt0 * N_MM, (kt0 + ch) * N_MM)
                bst = bstage.tile([P, ch, N_MM], bf16, name=f"bst{stage_tag}")
                mst = mstage.tile([P, ch, N_MM], i8, name=f"mst{stage_tag}")
                db = nc.sync.dma_start(out=bst[:], in_=base_d[rs, cs])
                nc.scalar.dma_start(out=mst[:], in_=mask_d[rs, cs])
                for j in range(ch):
                    nc.vector.scalar_tensor_tensor(
                        out=wp[:, kt0 + j, :],
                        in0=mst[:, j, :],
                        scalar=c128[:, 0:1],
                        in1=bst[:, j, :],
                        op0=mybir.AluOpType.mult,
                        op1=mybir.AluOpType.add,
                    )
                return db

            # Panel 0: 2-slab chunks, x slabs interleaved in consumption
            # order (even -> SWDGE, odd -> ACT ring between mask chunks).
            wp0 = wpool.tile([P, K_T, N_MM], bf16, name="wp")
            first_db = None
            for c in range(K_T // 2):
                db = build_chunk(wp0, 0, 2 * c, 2, "2")
                if first_db is None:
                    first_db = db
                anchor = first_db if c >= 2 else None
                x_slab(2 * c, nc.gpsimd, anchor=anchor)
                x_slab(2 * c + 1, nc.scalar, anchor=anchor)

            def build_panel(np_):
                wp = wpool.tile([P, K_T, N_MM], bf16, name="wp")
                for c in range(4):
                    build_chunk(wp, np_, 8 * c, 8, "8")
                return wp

            wps = {0: wp0, 1: build_panel(1)}
            for np_ in range(NPAN):
                wp = wps.pop(np_)
                # Panel 0: one 8-bank wave (DMA-paced). Last panel: 4/2/2
                # waves for a short drain tail. Middle: two 4-bank waves.
                if np_ == 0:
                    waves = [(0, M_T)]
                elif np_ == NPAN - 1:
                    waves = [(0, 4), (4, 2), (6, 2)]
                else:
                    waves = [(0, 4), (4, 4)]
                for w0, wlen in waves:
                    psums = {
                        m: mpsum.tile([P, N_MM], f32, name="mmps")
                        for m in range(w0, w0 + wlen)
                    }
                    for kt in range(K_T):
                        for m in range(w0, w0 + wlen):
                            nc.tensor.matmul(
                                psums[m][:],
                                lhsT=xts[:, kt, m * P:(m + 1) * P],
                                rhs=wp[:, kt, :],
                                start=(kt == 0),
                                stop=(kt == K_T - 1),
                            )
                    for i, m in enumerate(sorted(psums)):
                        ob = ostage.tile([P, N_MM], f32, name="ob")
                        if i % 2 == 0:
                            nc.scalar.copy(out=ob[:], in_=psums[m][:])
                        else:
                            nc.vector.tensor_copy(out=ob[:], in_=psums[m][:])
                        dma_eng = nc.sync if i % 2 == 0 else nc.scalar
                        dma_eng.dma_start(
                            out=out_d[
                                m * P:(m + 1) * P, np_ * N_MM:(np_ + 1) * N_MM
                            ],
                            in_=ob[:],
                        )
                if np_ + 2 < NPAN:
                    wps[np_ + 2] = build_panel(np_ + 2)

    nc.compile()
    return nc


_NC_CACHE = {}


def _get_nc():
    if "nc" not in _NC_CACHE:
        _NC_CACHE["nc"] = build_kernel()
    return _NC_CACHE["nc"]


def make_in_maps(x, base, coeff, mask):
    bf16 = ml_dtypes.bfloat16
    x2 = np.asarray(x, np.float32).reshape(FULL_M, FULL_K)

    # W inputs pretiled to [NPAN, P, K_T, N_MM] (p-major panels) so each
    # partition's panel data is one contiguous DMA line. Shared by all 8
    # cores (W is replicated).
    baseT = np.ascontiguousarray(
        np.asarray(base, np.float32).astype(bf16)
        .reshape(K_T, P, NPAN, N_MM).transpose(2, 1, 0, 3)
    ).reshape(NPAN * P, K_T * N_MM)
    maskT = np.ascontiguousarray(
        np.asarray(mask).astype(np.int8)
        .reshape(K_T, P, NPAN, N_MM).transpose(2, 1, 0, 3)
    ).reshape(NPAN * P, K_T * N_MM)
    c128 = np.full((P, 1), np.asarray(coeff, np.float32)[0], np.float32)

    in_maps = []
    for i in range(N_CORES):
        xc = x2[i * CORE_M:(i + 1) * CORE_M, :].astype(bf16)
        in_maps.append(
            {
                "xT": np.ascontiguousarray(xc.T),
                "baseT": baseT,
                "maskT": maskT,
                "coeff": c128,
            }
        )
    return in_maps, x.shape[:2]


def assemble(results, B, L):
    out = np.concatenate([results[i]["out"] for i in range(N_CORES)], axis=0)
    return out.reshape(B, L, FULL_N)


def kernel(x, base, coeff, mask):
    from concourse.bass_utils import run_bass_kernel_spmd

    in_maps, (B, L) = make_in_maps(x, base, coeff, mask)
    nc = _get_nc()
    res = run_bass_kernel_spmd(nc, in_maps, list(range(8)))
    return assemble(res.results, B, L)


# revision 11
# speedup vs baseline: 1.4587x; 1.0150x over previous
"""Trainium2 Bass kernel for nn_BinaryDiff: out = x @ base + coeff * (x @ mask).

Fused as a single matmul: out = x @ W where W = base + coeff * mask.

Sharding over 8 NeuronCores: data-parallel over rows — each core computes
1024 rows x 4096 cols of the [8192, 4096] output (x sharded by rows, W
replicated). x is pre-transposed/cast to bf16 on the host as part of the
sharding/layout prep, so the device program is a pure matmul pipeline:

  - x^T shard ([4096, 1024] bf16, 8 MiB) loaded once, resident in SBUF.
    Slabs alternate between the SWDGE ring and the ACT HWDGE ring
    (interleaved with panel-0 mask chunks in consumption order) — the
    SDMA engines round-robin across queues, so one queue alone gets only
    ~1/3 of HBM bandwidth and panel 0 would starve.
  - W streamed in eight 512-col panels, p-major host layout so DMA lines
    are contiguous per partition; base chunks on the SP HWDGE ring, mask
    chunks on the ACT HWDGE ring. Panel 0 uses 2-slab chunks (fast first
    arrival), later panels 8-slab chunks. Fused W = base + c*mask on DVE
    into a double-buffered bf16 panel cache, prefetched one panel ahead.
  - A short burst of warm-up matmuls on a memset tile runs while the
    first chunks land, so the PE HAM clock-gate is already at 8/8 when
    real work starts.
  - Panel 0 runs as one 8-bank wave (matmul consumption 1.73us/k-slab
    matches DMA delivery); later panels run two 4-bank waves rotating
    through the 8 PSUM banks so drains overlap the next wave's matmuls
    with no bank-handoff stall. The last panel drains in 4/2/2 waves to
    shorten the end-of-kernel DMA tail.
  - PSUM drained by alternating ScalarE/VectorE copies; fp32 out DMAs
    alternate between the two HWDGE rings.
"""

import numpy as np
import ml_dtypes

import concourse.mybir as mybir
import concourse.tile as tile
from concourse import bacc

P = 128
FULL_M, FULL_K, FULL_N = 8192, 4096, 4096
N_CORES = 8
CORE_M = FULL_M // N_CORES      # 1024 rows per core
K_T = FULL_K // P               # 32 k-slabs
M_T = CORE_M // P               # 8 m-tiles
NPAN = 8                        # W panels across N
N_MM = FULL_N // NPAN           # 512 (one PSUM bank)
N_WARM = 14                     # HAM warm-up matmuls


def build_kernel(debug=False):
    """Build the per-core Bass program. All cores run the same program (SPMD)."""
    f32 = mybir.dt.float32
    i8 = mybir.dt.int8
    bf16 = mybir.dt.bfloat16

    nc = bacc.Bacc("TRN2", target_bir_lowering=False, debug=debug)

    xT_d = nc.dram_tensor("xT", [FULL_K, CORE_M], bf16, kind="ExternalInput").ap()
    # base/mask pretiled on host to [NPAN, P, K_T, N_MM] (p-major panels,
    # flattened to 2D) so per-partition DMA lines are contiguous.
    base_d = nc.dram_tensor(
        "baseT", [NPAN * P, K_T * N_MM], bf16, kind="ExternalInput"
    ).ap()
    mask_d = nc.dram_tensor(
        "maskT", [NPAN * P, K_T * N_MM], i8, kind="ExternalInput"
    ).ap()
    coeff_d = nc.dram_tensor("coeff", [P, 1], f32, kind="ExternalInput").ap()
    out_d = nc.dram_tensor("out", [CORE_M, FULL_N], f32, kind="ExternalOutput").ap()

    with tile.TileContext(nc) as tc:
        with (
            tc.tile_pool(name="const", bufs=1) as const,
            tc.tile_pool(name="xts", bufs=1) as xpool,
            tc.tile_pool(name="bstage", bufs=4) as bstage,
            tc.tile_pool(name="mstage", bufs=4) as mstage,
            tc.tile_pool(name="wp", bufs=2) as wpool,
            tc.tile_pool(name="ostage", bufs=6) as ostage,
            tc.tile_pool(name="mpsum", bufs=8, space="PSUM") as mpsum,
        ):
            # HAM warm-up: PE runs junk matmuls at cold clock while the
            # first real operands stream in; by the time they land the
            # clock gate is open.
            wu = const.tile([P, 5 * P], bf16)
            nc.vector.memset(wu[:], 0.0)
            warm_ps = mpsum.tile([P, N_MM], f32, name="mmps")
            for _ in range(N_WARM):
                nc.tensor.matmul(
                    warm_ps[:], lhsT=wu[:, :P], rhs=wu[:, P:], start=True,
                    stop=True,
                )

            c128 = const.tile([P, 1], f32)
            nc.scalar.dma_start(out=c128[:], in_=coeff_d[:])

            xts = xpool.tile([P, K_T, CORE_M], bf16, name="xts")

            def x_slab(kt, eng, anchor=None):
                dx = eng.dma_start(
                    out=xts[:, kt, :], in_=xT_d[kt * P:(kt + 1) * P, :]
                )
                if anchor is not None:
                    tile.add_dep_helper(
                        dx.ins, anchor.ins, reason="pace x behind W chunk0"
                    )
                return dx

            def chunk_dma(np_, kt0, ch, stage_tag):
                """DMA one base/mask chunk [kt0, kt0+ch) of panel np_."""
                rs = slice(np_ * P, (np_ + 1) * P)
                cs = slice(kt0 * N_MM, (kt0 + ch) * N_MM)
                bst = bstage.tile([P, ch, N_MM], bf16, name=f"bst{stage_tag}")
                mst = mstage.tile([P, ch, N_MM], i8, name=f"mst{stage_tag}")
                db = nc.sync.dma_start(out=bst[:], in_=base_d[rs, cs])
                nc.scalar.dma_start(out=mst[:], in_=mask_d[rs, cs])
                return bst, mst, db

            def chunk_fuse(wp, kt0, ch, bst, mst):
                for j in range(ch):
                    nc.vector.scalar_tensor_tensor(
                        out=wp[:, kt0 + j, :],
                        in0=mst[:, j, :],
                        scalar=c128[:, 0:1],
                        in1=bst[:, j, :],
                        op0=mybir.AluOpType.mult,
                        op1=mybir.AluOpType.add,
                    )

            # Panel-0 prologue, in consumption order: 2-slab W chunks with
            # x slabs interleaved (even slabs -> SWDGE ring, odd -> SP ring
            # between base chunks; masks alone keep the ACT ring light).
            # W1's chunk DMAs are spread quarterly through the prologue so
            # panel 1 is staged early, but its DVE fuses are emitted after
            # panel 0's (DVE queue is strict FIFO — a W1 fuse emitted here
            # would block panel-0 fuses behind a W1 chunk still in flight).
            wp0 = wpool.tile([P, K_T, N_MM], bf16, name="wp")
            wp1 = wpool.tile([P, K_T, N_MM], bf16, name="wp")
            first_db = None
            w1_stage = []
            for c in range(K_T // 2):
                bst, mst, db = chunk_dma(0, 2 * c, 2, "2")
                if first_db is None:
                    first_db = db
                anchor = first_db if c >= 2 else None
                x_slab(2 * c, nc.gpsimd, anchor=anchor)
                x_slab(2 * c + 1, nc.sync, anchor=anchor)
                chunk_fuse(wp0, 2 * c, 2, bst, mst)
                if c % 4 == 3:
                    q = c // 4
                    w1_stage.append((8 * q, chunk_dma(1, 8 * q, 8, "8")))
            for kt0, (bst, mst, _) in w1_stage:
                chunk_fuse(wp1, kt0, 8, bst, mst)

            def build_panel(np_):
                wp = wpool.tile([P, K_T, N_MM], bf16, name="wp")
                for c in range(4):
                    bst, mst, _ = chunk_dma(np_, 8 * c, 8, "8")
                    chunk_fuse(wp, 8 * c, 8, bst, mst)
                return wp

            wps = {0: wp0, 1: wp1}
            for np_ in range(NPAN):
                wp = wps.pop(np_)
                # Panel 0: one 8-bank wave (DMA-paced). Last panel: 4/2/2
                # waves for a short drain tail. Middle: two 4-bank waves.
                if np_ == 0:
                    waves = [(0, M_T)]
                elif np_ == NPAN - 1:
                    waves = [(0, 4), (4, 2), (6, 1), (7, 1)]
                else:
                    waves = [(0, 4), (4, 4)]
                for w0, wlen in waves:
                    psums = {
                        m: mpsum.tile([P, N_MM], f32, name="mmps")
                        for m in range(w0, w0 + wlen)
                    }
                    for kt in range(K_T):
                        for m in range(w0, w0 + wlen):
                            nc.tensor.matmul(
                                psums[m][:],
                                lhsT=xts[:, kt, m * P:(m + 1) * P],
                                rhs=wp[:, kt, :],
                                start=(kt == 0),
                                stop=(kt == K_T - 1),
                            )
                    for i, m in enumerate(sorted(psums)):
                        ob = ostage.tile([P, N_MM], f32, name="ob")
                        if i % 2 == 0:
                            nc.scalar.copy(out=ob[:], in_=psums[m][:])
                        else:
                            nc.vector.tensor_copy(out=ob[:], in_=psums[m][:])
                        col0 = np_ * N_MM
                        if wlen == 1:
                            # end-of-kernel: halve the final DMAs across
                            # both HWDGE rings to shorten the drain tail
                            h = N_MM // 2
                            nc.sync.dma_start(
                                out=out_d[m * P:(m + 1) * P, col0:col0 + h],
                                in_=ob[:, :h],
                            )
                            nc.scalar.dma_start(
                                out=out_d[
                                    m * P:(m + 1) * P, col0 + h:col0 + N_MM
                                ],
                                in_=ob[:, h:],
                            )
                        else:
                            dma_eng = nc.sync if i % 2 == 0 else nc.scalar
                            dma_eng.dma_start(
                                out=out_d[
                                    m * P:(m + 1) * P, col0:col0 + N_MM
                                ],
                                in_=ob[:],
                            )
                if np_ + 2 < NPAN:
                    wps[np_ + 2] = build_panel(np_ + 2)

    nc.compile()
    return nc


_NC_CACHE = {}


def _get_nc():
    if "nc" not in _NC_CACHE:
        _NC_CACHE["nc"] = build_kernel()
    return _NC_CACHE["nc"]


def make_in_maps(x, base, coeff, mask):
    bf16 = ml_dtypes.bfloat16
    x2 = np.asarray(x, np.float32).reshape(FULL_M, FULL_K)

    # W inputs pretiled to [NPAN, P, K_T, N_MM] (p-major panels) so each
    # partition's panel data is one contiguous DMA line. Shared by all 8
    # cores (W is replicated).
    baseT = np.ascontiguousarray(
        np.asarray(base, np.float32).astype(bf16)
        .reshape(K_T, P, NPAN, N_MM).transpose(2, 1, 0, 3)
    ).reshape(NPAN * P, K_T * N_MM)
    maskT = np.ascontiguousarray(
        np.asarray(mask).astype(np.int8)
        .reshape(K_T, P, NPAN, N_MM).transpose(2, 1, 0, 3)
    ).reshape(NPAN * P, K_T * N_MM)
    c128 = np.full((P, 1), np.asarray(coeff, np.float32)[0], np.float32)

    in_maps = []
    for i in range(N_CORES):
        xc = x2[i * CORE_M:(i + 1) * CORE_M, :].astype(bf16)
        in_maps.append(
            {
                "xT": np.ascontiguousarray(xc.T),
                "baseT": baseT,
                "maskT": maskT,
                "coeff": c128,
            }
        )
    return in_maps, x.shape[:2]


def assemble(results, B, L):
    out = np.concatenate([results[i]["out"] for i in range(N_CORES)], axis=0)
    return out.reshape(B, L, FULL_N)


def kernel(x, base, coeff, mask):
    from concourse.bass_utils import run_bass_kernel_spmd

    in_maps, (B, L) = make_in_maps(x, base, coeff, mask)
    nc = _get_nc()
    res = run_bass_kernel_spmd(nc, in_maps, list(range(8)))
    return assemble(res.results, B, L)
